# revision 23
# baseline (speedup 1.0000x reference)
"""AdaptiveGraphDeformation Trainium kernel.

Data-parallel over 8 NeuronCores (2 samples each). Per sample on-device:
  1. 2x2 avg-pool via DVE row-pair adds + PE matmul against a 0.25-one-hot
     pooling matrix (produces pooled features transposed: [C, Lp]).
  2. MLP (192->64 gelu ->4) on PE, activations (gelu/tanh/softplus) on ACT.
  3. Deformed coords at pooled res; bilinear 64->128 upsample as small PE
     matmuls against static interpolation matrices. Two output layouts:
     [j, i] planes for bilinear weights, and wrapped+replicated planes
     generated directly by 8 sub-matmuls per channel so dma_gather's index
     layout needs no cross-partition shuffle.
  4. Per-pixel bilinear corner indices (int16) + weights (bf16) on DVE.
  5. Heavy gather: host pre-packs features as bf16 x-neighbor pairs
     [feat[p], feat[p_right]] so each bilinear corner pair is ONE 768B
     dma_gather descriptor; two gathers per pixel (top/bottom source rows).
  6. 4-term weighted combine (tensor_scalar + 3 scalar_tensor_tensor) on DVE.

Outputs are written in transposed/bf16 layouts for large DMA runs; host
un-transposes and upcasts.
"""

import numpy as np
import ml_dtypes

import concourse.bass as bass
import concourse.mybir as mybir
from concourse.tile import TileContext
from concourse.bass_utils import run_bass_kernel_spmd
from concourse import bacc

BF16 = ml_dtypes.bfloat16

B, H, W, C = 16, 128, 128, 192
HID = 64
HP, WP = H // 2, W // 2          # 64, 64
LP = HP * WP                     # 4096
L = H * W                        # 16384
NCORES = 8
SPB = B // NCORES                # samples per core = 2
R = 8                            # output rows per gather chunk (1024 idxs/call; >=2048 overflows the SWDGE ring)
NCHUNK = H // R
POOL_G = 8                       # hp-pairs per pooling group


def _build_consts():
    def upsample_mat():
        pos = np.arange(H, dtype=np.float32) * np.float32((HP - 1) / (H - 1))
        i0 = np.floor(pos).astype(np.int32)
        i1 = np.minimum(i0 + 1, HP - 1)
        w = (pos - i0.astype(np.float32)).astype(np.float32)
        U = np.zeros((HP, H), np.float32)
        U[i0, np.arange(H)] += 1.0 - w
        U[i1, np.arange(H)] += w
        return U

    U = upsample_mat()  # same for x and y (square problem)

    # Wrapped-replica upsample matrices: urep[jh][pw, p] = U[pw, 16*jh + p%16]
    urep = np.zeros((8, HP, 128), np.float32)
    for jh in range(8):
        urep[jh] = U[:, 16 * jh:16 * jh + 16][:, np.tile(np.arange(16), 8)]

    gx = 2.0 * np.arange(WP, dtype=np.float32) / np.float32(WP - 1) - 1.0
    gy = 2.0 * np.arange(HP, dtype=np.float32) / np.float32(HP - 1) - 1.0
    ox2d = np.broadcast_to(gx[None, :], (HP, WP))
    oy2d = np.broadcast_to(gy[:, None], (HP, WP))
    base2 = np.stack([ox2d.reshape(-1), oy2d.reshape(-1)]).astype(np.float32)

    qpool = np.zeros((W, WP), np.float32)
    qpool[np.arange(W), np.arange(W) // 2] = 0.25
    return U, urep, base2, qpool.astype(BF16)


class _LegalizedBacc(bacc.Bacc):
    """Bacc whose compile() also splits multi-wait instructions (this
    container's walrus rejects >1 inline sync-wait per instruction)."""

    def compile(self):
        super().compile()
        _legalize_waits(self)


def _legalize_waits(nc, max_waits=1):
    """This container's walrus build rejects instructions carrying more than
    one inline sync-wait; hoist overflow waits onto same-engine NoOps."""
    for fn in nc.m.functions:
        for bb in fn.blocks:
            new_list = []
            for ins in bb.instructions:
                si = ins.sync_info
                if si is not None and si.on_wait and len(si.on_wait) > max_waits:
                    waits = list(si.on_wait)
                    keep = waits[-max_waits:]
                    for k, w in enumerate(waits[:-max_waits]):
                        new_list.append(mybir.InstNoOp(
                            name=f"{ins.name}w{k}", engine=ins.engine,
                            ins=[], outs=[],
                            sync_info=mybir.SyncInfo(on_wait=[w], on_update=[]),
                        ))
                    si.on_wait = keep
                new_list.append(ins)
            bb.instructions[:] = new_list
    return nc


def _build_graph(legalize=True, act1=None, phases=3, debug=False):
    nc = _LegalizedBacc() if legalize else bass.Bass()
    f32, bf16 = mybir.dt.float32, mybir.dt.bfloat16
    i16, i32 = mybir.dt.int16, mybir.dt.int32
    AF = mybir.ActivationFunctionType
    _act1 = AF.Gelu if act1 is None else act1
    OP = mybir.AluOpType

    pairs = nc.dram_tensor("pairs", [SPB * L, 2 * C], bf16, kind="ExternalInput")
    featT = nc.dram_tensor("featT", [SPB, W, H * C], bf16, kind="ExternalInput")
    w1_d = nc.dram_tensor("w1", [C, HID], bf16, kind="ExternalInput")
    w2_d = nc.dram_tensor("w2", [HID, 34], bf16, kind="ExternalInput")
    b1_d = nc.dram_tensor("b1", [HID, 1], f32, kind="ExternalInput")
    sc2xy_d = nc.dram_tensor("sc2xy", [2, 1], f32, kind="ExternalInput")
    bi2xy_d = nc.dram_tensor("bi2xy", [2, 1], f32, kind="ExternalInput")
    sc2wh_d = nc.dram_tensor("sc2wh", [2, 1], f32, kind="ExternalInput")
    bi2wh_d = nc.dram_tensor("bi2wh", [2, 1], f32, kind="ExternalInput")
    u_d = nc.dram_tensor("umat", [HP, H], f32, kind="ExternalInput")
    urep_d = nc.dram_tensor("urep", [8, HP, 128], f32, kind="ExternalInput")
    base_d = nc.dram_tensor("base2", [2, LP], f32, kind="ExternalInput")
    qp_d = nc.dram_tensor("qpool", [W, WP], bf16, kind="ExternalInput")

    outT = nc.dram_tensor("outT", [SPB, W, H * C], bf16, kind="ExternalOutput")
    out_co = nc.dram_tensor("out_co", [SPB, 2, LP], f32, kind="ExternalOutput")
    out_pa = nc.dram_tensor("out_pa", [SPB, 4, LP], bf16, kind="ExternalOutput")

    scr_grid = nc.dram_tensor("scr_grid", [SPB, 2, LP], f32, kind="Internal")
    if debug:
        dbg_idx = nc.dram_tensor("dbg_idx", [SPB, 2, 128, H * 8], i16,
                                 kind="ExternalOutput")
        dbg_g = nc.dram_tensor("dbg_g", [SPB, 2, 128, R * 2 * C], bf16,
                               kind="ExternalOutput")
        dbg_w = nc.dram_tensor("dbg_w", [SPB, 4, W, H], f32,
                               kind="ExternalOutput")

    with TileContext(nc) as tc:
        with (
            tc.tile_pool(name="consts", bufs=1) as cpool,
            tc.tile_pool(name="work", bufs=2) as wpool,
            tc.tile_pool(name="stat", bufs=1) as spool,
            tc.tile_pool(name="acc", bufs=4) as apool,
            tc.tile_pool(name="gath", bufs=2) as gpool,
            tc.tile_pool(name="idxp", bufs=1) as ipool,
            tc.tile_pool(name="psum", bufs=2, space="PSUM") as ppool,
            tc.tile_pool(name="psumS", bufs=1, space="PSUM") as ppoolS,
            tc.tile_pool(name="psumU", bufs=2, space="PSUM") as ppoolU,
        ):
            w1a = cpool.tile([128, HID], bf16, tag="w1a")
            nc.sync.dma_start(out=w1a[:], in_=w1_d[0:128, :])
            w1b = cpool.tile([64, HID], bf16, tag="w1b")
            nc.sync.dma_start(out=w1b[:], in_=w1_d[128:192, :])
            w2s = cpool.tile([HID, 34], bf16, tag="w2")
            nc.sync.dma_start(out=w2s[:], in_=w2_d[:])
            b1s = cpool.tile([HID, 1], f32, tag="b1")
            nc.sync.dma_start(out=b1s[:], in_=b1_d[:])
            sc2xy = cpool.tile([2, 1], f32, tag="sc2xy")
            nc.sync.dma_start(out=sc2xy[:], in_=sc2xy_d[:])
            bi2xy = cpool.tile([2, 1], f32, tag="bi2xy")
            nc.sync.dma_start(out=bi2xy[:], in_=bi2xy_d[:])
            sc2wh = cpool.tile([2, 1], f32, tag="sc2wh")
            nc.sync.dma_start(out=sc2wh[:], in_=sc2wh_d[:])
            bi2wh = cpool.tile([2, 1], f32, tag="bi2wh")
            nc.sync.dma_start(out=bi2wh[:], in_=bi2wh_d[:])
            umat = cpool.tile([HP, H], f32, tag="umat")
            nc.sync.dma_start(out=umat[:], in_=u_d[:])
            urep = cpool.tile([HP, 8, 128], f32, tag="urep")
            nc.sync.dma_start(out=urep[:],
                              in_=urep_d.rearrange("jh pw p -> pw jh p"))
            base2 = cpool.tile([2, LP], f32, tag="base2")
            nc.sync.dma_start(out=base2[:], in_=base_d[:])
            qpool_s = cpool.tile([W, WP], bf16, tag="qpool")
            nc.sync.dma_start(out=qpool_s[:], in_=qp_d[:])

            for s in range(SPB):
                # ==== phase 1a: pooling -> pooled_T (bf16) =============
                pooledTa = spool.tile([128, LP], bf16, tag="pooledTa")
                pooledTb = spool.tile([64, LP], bf16, tag="pooledTb")
                for g in range(HP // POOL_G):
                    vin = wpool.tile([W, POOL_G * 2 * C], bf16, tag="vin")
                    nc.sync.dma_start(
                        out=vin[:],
                        in_=featT[s, :, g * POOL_G * 2 * C:(g + 1) * POOL_G * 2 * C],
                    )
                    ppa = ppool.tile([128, POOL_G * WP], f32, tag="poolA")
                    ppb = ppool.tile([64, POOL_G * WP], f32, tag="poolB")
                    for k in range(POOL_G):
                        vsum = apool.tile([W, C], bf16, tag="vsum")
                        nc.vector.tensor_tensor(
                            out=vsum[:],
                            in0=vin[:, (2 * k) * C:(2 * k + 1) * C],
                            in1=vin[:, (2 * k + 1) * C:(2 * k + 2) * C],
                            op=OP.add,
                        )
                        nc.tensor.matmul(
                            ppa[:, k * WP:(k + 1) * WP],
                            vsum[:, 0:128], qpool_s[:],
                            start=True, stop=True,
                        )
                        nc.tensor.matmul(
                            ppb[:, k * WP:(k + 1) * WP],
                            vsum[:, 128:192], qpool_s[:],
                            start=True, stop=True,
                        )
                    nc.scalar.activation(
                        out=pooledTa[:, g * POOL_G * WP:(g + 1) * POOL_G * WP],
                        in_=ppa[:], func=AF.Copy)
                    nc.scalar.activation(
                        out=pooledTb[:, g * POOL_G * WP:(g + 1) * POOL_G * WP],
                        in_=ppb[:], func=AF.Copy)

                # ==== phase 1b: MLP -> delta [4, LP] bf16 ==============
                delta_xy = spool.tile([2, LP], bf16, tag="delta_xy")
                delta_wh = spool.tile([2, LP], bf16, tag="delta_wh")
                for t in range(LP // 128):
                    sl = slice(t * 128, (t + 1) * 128)
                    hps = ppoolS.tile([HID, 128], f32, tag="hps")
                    nc.tensor.matmul(hps[:], w1a[:], pooledTa[:, sl],
                                     start=True, stop=False)
                    nc.tensor.matmul(hps[:], w1b[:], pooledTb[:, sl],
                                     start=False, stop=True)
                    hsb = apool.tile([HID, 128], bf16, tag="hsb")
                    nc.scalar.activation(out=hsb[:], in_=hps[:], func=_act1,
                                         bias=b1s[:], scale=1.0)
                    dps = ppoolS.tile([34, 128], f32, tag="dps")
                    nc.tensor.matmul(dps[:], w2s[:], hsb[:], start=True, stop=True)
                    nc.scalar.activation(out=delta_xy[:, sl], in_=dps[0:2, :],
                                         func=AF.Tanh, bias=bi2xy[:],
                                         scale=sc2xy[:])
                    esp = apool.tile([2, 128], f32, tag="esp")
                    nc.scalar.activation(out=esp[:], in_=dps[32:34, :],
                                         func=AF.Exp, bias=bi2wh[:],
                                         scale=sc2wh[:])
                    nc.scalar.activation(out=delta_wh[:, sl], in_=esp[:],
                                         func=AF.Ln, bias=1.0, scale=1.0)
                nc.sync.dma_start(out=out_pa[s, 0:2], in_=delta_xy[:])
                nc.sync.dma_start(out=out_pa[s, 2:4], in_=delta_wh[:])

                # ==== coords at pooled res =============================
                cn = spool.tile([2, LP], f32, tag="cn")
                nc.vector.scalar_tensor_tensor(
                    out=cn[:], in0=delta_xy[:], scalar=float(2.0 / WP),
                    in1=base2[:], op0=OP.mult, op1=OP.add)
                nc.vector.tensor_scalar(out=cn[:], in0=cn[:], scalar1=1.0,
                                        scalar2=-1.0, op0=OP.min, op1=OP.max)
                nc.sync.dma_start(out=out_co[s], in_=cn[:])
                nc.vector.tensor_scalar(out=cn[:], in0=cn[:],
                                        scalar1=float((W - 1) / 2.0),
                                        scalar2=float((W - 1) / 2.0),
                                        op0=OP.mult, op1=OP.add)
                nc.sync.dma_start(out=scr_grid[s], in_=cn[:])

                # ==== upsample to full res =============================
                planes = []      # [j, i] planes (for weights)
                wplanes = []     # wrapped+replicated planes (for indices)
                for ch in range(2):
                    grid = apool.tile([HP, WP], f32, tag="grid")
                    nc.sync.dma_start(
                        out=grid[:],
                        in_=scr_grid[s, ch].rearrange("(ph pw) -> ph pw", ph=HP))
                    t1p = ppoolU.tile([WP, H], f32, tag="ups")
                    nc.tensor.matmul(t1p[:], grid[:], umat[:], start=True, stop=True)
                    t1s = apool.tile([WP, H], f32, tag="t1s")
                    nc.vector.tensor_copy(t1s[:], t1p[:])
                    pxp = ppoolU.tile([W, H], f32, tag="ups")
                    nc.tensor.matmul(pxp[:], umat[:], t1s[:], start=True, stop=True)
                    pxs = wpool.tile([W, H], f32, tag=f"plane{ch}")
                    nc.vector.tensor_copy(pxs[:], pxp[:])
                    planes.append(pxs)

                    wrep = spool.tile([128, H, 8], f32, tag=f"wrep{ch}")
                    for jh in range(8):
                        wpp = ppoolU.tile([128, H], f32, tag="ups")
                        nc.tensor.matmul(wpp[:], urep[:, jh, :], t1s[:],
                                         start=True, stop=True)
                        nc.scalar.activation(out=wrep[:, :, jh], in_=wpp[:],
                                             func=AF.Copy)
                    wplanes.append(wrep)
                ixT, iyT = planes
                wxr, wyr = wplanes

                # ==== weights (on [j, i] planes) =======================
                ixi = apool.tile([W, H], i32, tag="ixi")
                nc.vector.tensor_scalar(out=ixi[:], in0=ixT[:],
                                        scalar1=0.49999997, scalar2=None,
                                        op0=OP.subtract)
                ix0p = apool.tile([W, H], f32, tag="ix0p")
                nc.vector.tensor_copy(ix0p[:], ixi[:])
                fx = apool.tile([W, H], f32, tag="fx")
                nc.vector.tensor_tensor(out=fx[:], in0=ixT[:], in1=ix0p[:],
                                        op=OP.subtract)
                iyi = apool.tile([W, H], i32, tag="iyi")
                nc.vector.tensor_scalar(out=iyi[:], in0=iyT[:],
                                        scalar1=0.49999997, scalar2=None,
                                        op0=OP.subtract)
                iy0p = apool.tile([W, H], f32, tag="iy0p")
                nc.vector.tensor_copy(iy0p[:], iyi[:])
                fy = apool.tile([W, H], f32, tag="fy")
                nc.vector.tensor_tensor(out=fy[:], in0=iyT[:], in1=iy0p[:],
                                        op=OP.subtract)
                gx_ = apool.tile([W, H], f32, tag="gx_")
                nc.vector.tensor_scalar(out=gx_[:], in0=fx[:], scalar1=-1.0,
                                        scalar2=1.0, op0=OP.mult, op1=OP.add)
                gy_ = apool.tile([W, H], f32, tag="gy_")
                nc.vector.tensor_scalar(out=gy_[:], in0=fy[:], scalar1=-1.0,
                                        scalar2=1.0, op0=OP.mult, op1=OP.add)
                w00 = wpool.tile([W, H], f32, tag="w00")
                nc.vector.tensor_tensor(out=w00[:], in0=gx_[:], in1=gy_[:], op=OP.mult)
                w01 = wpool.tile([W, H], f32, tag="w01")
                nc.vector.tensor_tensor(out=w01[:], in0=fx[:], in1=gy_[:], op=OP.mult)
                w10 = wpool.tile([W, H], f32, tag="w10")
                nc.vector.tensor_tensor(out=w10[:], in0=gx_[:], in1=fy[:], op=OP.mult)
                w11 = wpool.tile([W, H], f32, tag="w11")
                nc.vector.tensor_tensor(out=w11[:], in0=fx[:], in1=fy[:], op=OP.mult)

                # ==== indices (wrapped planes -> int16) ================
                wxr_f = wxr[:].rearrange("p a b -> p (a b)")
                wyr_f = wyr[:].rearrange("p a b -> p (a b)")
                ixwi = ipool.tile([128, H * 8], i32, tag="iA")
                nc.vector.tensor_scalar(out=ixwi[:], in0=wxr_f,
                                        scalar1=0.49999997, scalar2=None,
                                        op0=OP.subtract)
                ix0w = ipool.tile([128, H * 8], f32, tag="iB")
                nc.vector.tensor_copy(ix0w[:], ixwi[:])
                iywi = ipool.tile([128, H * 8], i32, tag="iA")
                nc.vector.tensor_scalar(out=iywi[:], in0=wyr_f,
                                        scalar1=0.49999997, scalar2=None,
                                        op0=OP.subtract)
                iy0w = ipool.tile([128, H * 8], f32, tag="iC")
                nc.vector.tensor_copy(iy0w[:], iywi[:])
                iybw = ipool.tile([128, H * 8], f32, tag="iA")
                nc.vector.tensor_scalar(out=iybw[:], in0=iy0w[:], scalar1=1.0,
                                        scalar2=float(H - 1), op0=OP.add,
                                        op1=OP.min)
                topf = ipool.tile([128, H * 8], f32, tag="iD")
                nc.vector.scalar_tensor_tensor(
                    out=topf[:], in0=iy0w[:], scalar=float(W), in1=ix0w[:],
                    op0=OP.mult, op1=OP.add)
                botf = ipool.tile([128, H * 8], f32, tag="iC")
                nc.vector.scalar_tensor_tensor(
                    out=botf[:], in0=iybw[:], scalar=float(W), in1=ix0w[:],
                    op0=OP.mult, op1=OP.add)
                idx_t = wpool.tile([128, H * 8], i16, tag="idx_t")
                nc.vector.tensor_copy(idx_t[:], topf[:])
                idx_b = wpool.tile([128, H * 8], i16, tag="idx_b")
                nc.vector.tensor_copy(idx_b[:], botf[:])
                if debug:
                    nc.sync.dma_start(out=dbg_idx[s, 0], in_=idx_t[:])
                    nc.sync.dma_start(out=dbg_idx[s, 1], in_=idx_b[:])
                    nc.sync.dma_start(out=dbg_w[s, 0], in_=w00[:])
                    nc.sync.dma_start(out=dbg_w[s, 1], in_=w01[:])
                    nc.sync.dma_start(out=dbg_w[s, 2], in_=w10[:])
                    nc.sync.dma_start(out=dbg_w[s, 3], in_=w11[:])

                # ==== gather + combine =================================
                if phases < 2:
                    continue
                pairs_v = pairs[s * L:(s + 1) * L, :]
                for chk in range(NCHUNK if phases >= 3 else 1):
                    i0 = chk * R
                    gtop = gpool.tile([128, R, 2 * C], bf16, tag="gtop")
                    nc.gpsimd.dma_gather(
                        out_ap=gtop[:], in_ap=pairs_v,
                        idxs_ap=idx_t[:, i0 * 8:(i0 + R) * 8],
                        num_idxs=R * 128, num_idxs_reg=R * 128,
                        elem_size=2 * C, elem_step=2 * C)
                    gbot = gpool.tile([128, R, 2 * C], bf16, tag="gbot")
                    nc.gpsimd.dma_gather(
                        out_ap=gbot[:], in_ap=pairs_v,
                        idxs_ap=idx_b[:, i0 * 8:(i0 + R) * 8],
                        num_idxs=R * 128, num_idxs_reg=R * 128,
                        elem_size=2 * C, elem_step=2 * C)

                    if debug and chk == 0:
                        nc.sync.dma_start(
                            out=dbg_g[s, 0],
                            in_=gtop[:].rearrange("p a b -> p (a b)"))
                        nc.sync.dma_start(
                            out=dbg_g[s, 1],
                            in_=gbot[:].rearrange("p a b -> p (a b)"))
                    if phases == 5:
                        continue
                    outt = gpool.tile([128, R * C], bf16, tag="outt")
                    for r in range(R):
                        i = i0 + r
                        a1 = apool.tile([128, C], bf16, tag="a1")
                        nc.vector.tensor_scalar(
                            out=a1[:], in0=gtop[:, r, 0:C],
                            scalar1=w00[:, i:i + 1], scalar2=None, op0=OP.mult)
                        a2 = apool.tile([128, C], bf16, tag="a2")
                        nc.vector.scalar_tensor_tensor(
                            out=a2[:], in0=gtop[:, r, C:2 * C],
                            scalar=w01[:, i:i + 1], in1=a1[:],
                            op0=OP.mult, op1=OP.add)
                        a3 = apool.tile([128, C], bf16, tag="a3")
                        nc.vector.scalar_tensor_tensor(
                            out=a3[:], in0=gbot[:, r, 0:C],
                            scalar=w10[:, i:i + 1], in1=a2[:],
                            op0=OP.mult, op1=OP.add)
                        nc.vector.scalar_tensor_tensor(
                            out=outt[:, r * C:(r + 1) * C], in0=gbot[:, r, C:2 * C],
                            scalar=w11[:, i:i + 1], in1=a3[:],
                            op0=OP.mult, op1=OP.add)
                    nc.sync.dma_start(
                        out=outT[s, :, i0 * C:(i0 + R) * C], in_=outt[:])
    if legalize:
        nc.finalize()
    return nc


_GRAPH_CACHE = {}


def _get_graph():
    if "nc" not in _GRAPH_CACHE:
        _GRAPH_CACHE["nc"] = _build_graph()
    return _GRAPH_CACHE["nc"]


def kernel(features, H=None, W=None, W1=None, b1=None, W2=None, b2=None,
           weight_coef=None, **kw):
    features = np.asarray(features, dtype=np.float32)
    W1 = np.asarray(W1, np.float32)
    b1 = np.asarray(b1, np.float32)
    W2 = np.asarray(W2, np.float32)
    b2 = np.asarray(b2, np.float32)
    weight_coef = np.asarray(weight_coef, np.float32)

    U, urep, base2, qpool = _build_consts()
    W2pad = np.zeros((HID, 34), np.float32)
    W2pad[:, 0:2] = W2[:, 0:2]
    W2pad[:, 32:34] = W2[:, 2:4]

    feat = features.reshape(B, 128, 128, C)
    fb = feat.astype(BF16)
    right = fb[:, :, list(range(1, 128)) + [127], :]
    pairs = np.concatenate([fb, right], axis=3).reshape(B, L, 2 * C)
    featT = np.ascontiguousarray(fb.transpose(0, 2, 1, 3)).reshape(B, 128, 128 * C)

    consts = {
        "w1": W1.astype(BF16), "w2": W2pad.astype(BF16),
        "b1": b1.reshape(HID, 1).astype(np.float32),
        "sc2xy": weight_coef[0:2].reshape(2, 1).astype(np.float32),
        "bi2xy": (b2 * weight_coef)[0:2].reshape(2, 1).astype(np.float32),
        "sc2wh": weight_coef[2:4].reshape(2, 1).astype(np.float32),
        "bi2wh": (b2 * weight_coef)[2:4].reshape(2, 1).astype(np.float32),
        "umat": U, "urep": urep, "base2": base2, "qpool": qpool,
    }
    in_maps = []
    for core in range(NCORES):
        s0 = core * SPB
        m = dict(consts)
        m["pairs"] = np.ascontiguousarray(pairs[s0:s0 + SPB]).reshape(SPB * L, 2 * C)
        m["featT"] = np.ascontiguousarray(featT[s0:s0 + SPB])
        in_maps.append(m)

    nc = _get_graph()
    res = run_bass_kernel_spmd(nc, in_maps, core_ids=list(range(NCORES)))
    _GRAPH_CACHE["last_result"] = res

    outf = np.empty((B, 128, 128, C), np.float32)
    outc = np.empty((B, LP, 2), np.float32)
    outp = np.empty((B, LP, 4), np.float32)
    for core in range(NCORES):
        r = res.results[core]
        s0 = core * SPB
        o = r["outT"].reshape(SPB, 128, 128, C).astype(np.float32)
        outf[s0:s0 + SPB] = o.transpose(0, 2, 1, 3)
        outc[s0:s0 + SPB] = r["out_co"].transpose(0, 2, 1)
        outp[s0:s0 + SPB] = r["out_pa"].astype(np.float32).transpose(0, 2, 1)
    return (outf.reshape(B, L, C), outp, outc)
